# revision 7
# baseline (speedup 1.0000x reference)
"""DiffAttention kernel for 8 TRN2 NeuronCores (Bass/Tile).

Reference computation (see problem): x [1,128,32,32,32] is stride-2
subsampled to xs [128, N=4096 tokens]; qkv = w_qkv @ xs per head
(4 heads, head_dim 32, split into two halves of 16 for the two
softmaxes); diff_attn = softmax(q1k1) - 0.1*softmax(q2k2); out = diff
attn @ v, reshaped back to [1,128,16,16,16].

Sharding: tensor-parallel over (head, query-half) = 8 shards, one per
core. Each core computes its head's full K/V over all 4096 tokens and
attention for its 2048 queries.

Per-core dataflow (all on-chip, flash-style, no NxN HBM traffic):
  - k1,k2 / q1,q2 projections via PE with col-tiled placement so that
    strip 1 (partitions 32:48) holds the (q1,k1) pair and strip 2
    (partitions 64:80) holds (q2,k2); scores are computed TRANSPOSED,
    sT[m,n] = k^T q, so the softmax denominator can be folded into the
    AV matmul via a ones-column appended to v^T (no extra PE streams).
  - exp on ACT directly from PSUM (scale folded into the activation).
  - AV: out^T[d,n] accumulated over m-chunks in PSUM; AV1 at psum
    partitions 0:33, AV2 at 64:97 (col-tiled, run concurrently).
  - finalize: PE-transpose av -> [n,33], per-partition reciprocal of
    the sum column, combine out = av1/s1 - 0.1*av2/s2 on DVE.
"""

import numpy as np
import ml_dtypes

import concourse.bass as bass
import concourse.mybir as mybir
import concourse.tile as tile
from concourse import bacc
from concourse.bass import ts, ds
from concourse.bass_utils import run_bass_kernel_spmd

BF16 = mybir.dt.bfloat16
F32 = mybir.dt.float32
NP_BF16 = ml_dtypes.bfloat16

C = 128          # channels
HEADS = 4
HD = 32          # head_dim
DH = 16          # d_half
LAMBDA = 0.1
SCALE = HD ** -0.5
R = 2
N_CORES = 8
N = 4096         # tokens after subsample
NQ = N // 2      # queries per core

# weight tensor column layout (w input, [128, 96]):
WV = slice(0, 32)     # w_v^T   (rhs of vT matmuls)
WK1 = slice(32, 48)   # w_k1^T
WK2 = slice(48, 64)   # w_k2^T
WQ1 = slice(64, 80)   # w_q1^T
WQ2 = slice(80, 96)   # w_q2^T


def build_nc(NT=N, NQL=NQ, NBS=1024):
    """Build the SPMD Bass program for one core = (head, query-half).

    Per-core inputs:
      xs    [128, NT]   bf16  all tokens, channel-major (for K and V)
      xq    [128, NQL]  bf16  this core's query tokens
      w     [128, 96]   bf16  columns per WV/WK1/WK2/WQ1/WQ2 slices
      ident [128, 33]   f32   identity blocks at partitions 0:33, 64:97
    Output:
      out   [NQL, 32]   f32   attention output (n, d) for the queries
    """
    assert NT % 512 == 0 and NQL % NBS == 0 and NBS % 512 == 0
    assert NQL % 128 == 0
    MC = NT // 128        # m-chunks (key/value chunks of 128 tokens)
    NC128 = NQL // 128    # query chunks of 128 for the finalize
    Exp = mybir.ActivationFunctionType.Exp

    nc = bacc.Bacc()
    xs_d = nc.declare_dram_parameter("xs", [C, NT], BF16, isOutput=False)
    xq_d = nc.declare_dram_parameter("xq", [C, NQL], BF16, isOutput=False)
    w_d = nc.declare_dram_parameter("w", [C, 96], BF16, isOutput=False)
    id_d = nc.declare_dram_parameter("ident", [C, 33], F32, isOutput=False)
    out_d = nc.declare_dram_parameter("out", [NQL, HD], F32, isOutput=True)

    with tile.TileContext(nc) as tc:
        with (
            tc.tile_pool(name="consts", bufs=1) as consts,
            tc.tile_pool(name="mains", bufs=1) as mains,
        ):
            xs_sb = mains.tile([C, NT], BF16)
            nc.sync.dma_start(out=xs_sb[:, :], in_=xs_d[:, :])
            xq_sb = mains.tile([C, NQL], BF16)
            nc.sync.dma_start(out=xq_sb[:, :], in_=xq_d[:, :])
            w_sb = consts.tile([C, 96], BF16)
            nc.sync.dma_start(out=w_sb[:, :], in_=w_d[:, :])
            id_sb = consts.tile([C, 33], F32)
            nc.sync.dma_start(out=id_sb[:, :], in_=id_d[:, :])

            kk_sb = mains.tile([C, NT], BF16)    # parts 32:48 k1, 64:80 k2
            qq_sb = mains.tile([C, NQL], BF16)   # parts 32:48 q1, 64:80 q2
            vTa_sb = mains.tile([C, MC * 33], BF16)  # per chunk: v^T | ones
            av_sb = mains.tile([C, NQL], F32)    # parts 0:33 AV1|s1, 64:97 AV2|s2
            out_sb = mains.tile([C, NC128 * HD], F32)

            nc.vector.memset(vTa_sb[:, :], 1.0)

            # --- projections: k1,k2 (all tokens), q1,q2 (queries), vT ---
            with tc.tile_pool(name="pre_ps", bufs=2, space="PSUM") as prepool:
                for t in range(NT // 512):
                    ps_kv = prepool.tile([C, 512], F32, tag="ps_kv")
                    nc.tensor.matmul(ps_kv[32:48, :], lhsT=w_sb[:, WK1],
                                     rhs=xs_sb[:, ts(t, 512)], start=True, stop=True)
                    nc.tensor.matmul(ps_kv[64:80, :], lhsT=w_sb[:, WK2],
                                     rhs=xs_sb[:, ts(t, 512)], start=True, stop=True)
                    nc.vector.tensor_copy(kk_sb[32:48, ts(t, 512)], ps_kv[32:48, :])
                    nc.vector.tensor_copy(kk_sb[64:80, ts(t, 512)], ps_kv[64:80, :])
                for t in range(NQL // 512):
                    ps_q = prepool.tile([C, 512], F32, tag="ps_q")
                    nc.tensor.matmul(ps_q[32:48, :], lhsT=w_sb[:, WQ1],
                                     rhs=xq_sb[:, ts(t, 512)], start=True, stop=True)
                    nc.tensor.matmul(ps_q[64:80, :], lhsT=w_sb[:, WQ2],
                                     rhs=xq_sb[:, ts(t, 512)], start=True, stop=True)
                    nc.vector.tensor_copy(qq_sb[32:48, ts(t, 512)], ps_q[32:48, :])
                    nc.vector.tensor_copy(qq_sb[64:80, ts(t, 512)], ps_q[64:80, :])
                for m in range(MC):
                    ps_vt = prepool.tile([C, HD], F32, tag="ps_vt")
                    nc.tensor.matmul(ps_vt[:, :], lhsT=xs_sb[:, ts(m, 128)],
                                     rhs=w_sb[:, WV], start=True, stop=True)
                    nc.vector.tensor_copy(vTa_sb[:, ds(m * 33, HD)], ps_vt[:, :])

            # --- main attention loop ---
            with (
                tc.tile_pool(name="s_ps", bufs=1, space="PSUM") as spool,
                tc.tile_pool(name="av_ps", bufs=2, space="PSUM") as avpool,
                tc.tile_pool(name="e_sb", bufs=2) as epool,
            ):
                for nb in range(NQL // NBS):
                    av_ps = avpool.tile([C, NBS], F32, tag="av")
                    for m in range(MC):
                        s1_ps = spool.tile([C, NBS], F32, tag="s1")
                        s2_ps = spool.tile([C, NBS], F32, tag="s2")
                        for j in range(NBS // 512):
                            nsl = ds(nb * NBS + j * 512, 512)
                            nc.tensor.matmul(s1_ps[:, ts(j, 512)],
                                             lhsT=kk_sb[32:48, ts(m, 128)],
                                             rhs=qq_sb[32:48, nsl],
                                             start=True, stop=True)
                            nc.tensor.matmul(s2_ps[:, ts(j, 512)],
                                             lhsT=kk_sb[64:80, ts(m, 128)],
                                             rhs=qq_sb[64:80, nsl],
                                             start=True, stop=True)
                        e1_sb = epool.tile([C, NBS], BF16, tag="e1")
                        e2_sb = epool.tile([C, NBS], BF16, tag="e2")
                        nc.scalar.activation(e1_sb[:, :], s1_ps[:, :], Exp, scale=SCALE)
                        nc.scalar.activation(e2_sb[:, :], s2_ps[:, :], Exp, scale=SCALE)
                        first, last = (m == 0), (m == MC - 1)
                        for j in range(NBS // 512):
                            # av1 (parts 0:33) and av2 (64:97) share psum
                            # banks on disjoint partitions; the sim's group
                            # check is partition-unaware, hence the skip.
                            nc.tensor.matmul(av_ps[0:33, ts(j, 512)],
                                             lhsT=vTa_sb[:, ds(m * 33, 33)],
                                             rhs=e1_sb[:, ts(j, 512)],
                                             start=first, stop=last,
                                             skip_group_check=True)
                            nc.tensor.matmul(av_ps[64:97, ts(j, 512)],
                                             lhsT=vTa_sb[:, ds(m * 33, 33)],
                                             rhs=e2_sb[:, ts(j, 512)],
                                             start=first, stop=last,
                                             skip_group_check=True)
                    nc.vector.tensor_copy(av_sb[0:33, ds(nb * NBS, NBS)], av_ps[0:33, :])
                    nc.vector.tensor_copy(av_sb[64:97, ds(nb * NBS, NBS)], av_ps[64:97, :])

            # --- finalize: transpose to [n, 33], normalize, combine ---
            with (
                tc.tile_pool(name="fin_ps", bufs=1, space="PSUM") as fpool,
                tc.tile_pool(name="fin_sb", bufs=1) as fsb,
            ):
                # per query chunk c: psT1[:, c*64 : c*64+33] = av1^T chunk
                # (33 cols at a 64 stride so no matmul output crosses a
                # PSUM bank boundary)
                psT1 = fpool.tile([C, NC128 * 64], F32, tag="psT1")
                psT2 = fpool.tile([C, NC128 * 64], F32, tag="psT2")
                for cq in range(NC128):
                    nc.tensor.transpose(psT1[:, ds(cq * 64, 33)],
                                        av_sb[0:33, ts(cq, 128)], id_sb[0:33, :])
                    nc.tensor.transpose(psT2[:, ds(cq * 64, 33)],
                                        av_sb[64:97, ts(cq, 128)], id_sb[64:97, :])
                r1_sb = fsb.tile([C, NC128], F32)
                r2_sb = fsb.tile([C, NC128], F32)
                sum1_view = psT1[:, :].rearrange("p (c x) -> p c x", x=64)[:, :, 32:33]
                sum2_view = psT2[:, :].rearrange("p (c x) -> p c x", x=64)[:, :, 32:33]
                nc.vector.reciprocal(r1_sb[:, :, None], sum1_view)
                nc.vector.reciprocal(r2_sb[:, :, None], sum2_view)
                nc.vector.tensor_scalar_mul(r2_sb[:, :], r2_sb[:, :], -LAMBDA)
                o1_sb = fsb.tile([C, NC128 * HD], F32)
                o2_sb = fsb.tile([C, NC128 * HD], F32)
                av1t_view = psT1[:, :].rearrange("p (c x) -> p c x", x=64)[:, :, 0:32]
                av2t_view = psT2[:, :].rearrange("p (c x) -> p c x", x=64)[:, :, 0:32]
                o1_view = o1_sb[:, :].rearrange("p (c d) -> p c d", d=HD)
                o2_view = o2_sb[:, :].rearrange("p (c d) -> p c d", d=HD)
                nc.vector.tensor_tensor(
                    o1_view, av1t_view,
                    r1_sb[:, :, None].to_broadcast((C, NC128, HD)),
                    mybir.AluOpType.mult)
                nc.vector.tensor_tensor(
                    o2_view, av2t_view,
                    r2_sb[:, :, None].to_broadcast((C, NC128, HD)),
                    mybir.AluOpType.mult)
                nc.vector.tensor_tensor(
                    out_sb[:, :], o1_sb[:, :], o2_sb[:, :], mybir.AluOpType.add)

            nc.sync.dma_start(
                out=out_d[:, :].rearrange("(c p) d -> p c d", p=C),
                in_=out_sb[:, :].rearrange("p (c d) -> p c d", d=HD),
            )
    nc.compile()
    return nc


def make_identity_input():
    ident = np.zeros((C, 33), np.float32)
    ident[0:33, :] = np.eye(33, dtype=np.float32)
    ident[64:97, :] = np.eye(33, dtype=np.float32)
    return ident


def make_in_maps(x, w_qkv):
    """Host-side sharding: subsample, pack per-core inputs."""
    xs = np.ascontiguousarray(x[0][:, ::R, ::R, ::R]).reshape(C, N)
    xs_b = xs.astype(NP_BF16)
    ident = make_identity_input()
    in_maps = []
    for core in range(N_CORES):
        h, half = divmod(core, 2)
        wq = w_qkv[h * 96: h * 96 + 32]       # [32, 128]
        wk = w_qkv[h * 96 + 32: h * 96 + 64]
        wv = w_qkv[h * 96 + 64: h * 96 + 96]
        w = np.empty((C, 96), np.float32)
        w[:, WV] = wv.T
        w[:, WK1] = wk[0:DH].T
        w[:, WK2] = wk[DH:HD].T
        w[:, WQ1] = wq[0:DH].T
        w[:, WQ2] = wq[DH:HD].T
        in_maps.append({
            "xs": xs_b,
            "xq": np.ascontiguousarray(xs_b[:, half * NQ:(half + 1) * NQ]),
            "w": w.astype(NP_BF16),
            "ident": ident,
        })
    return in_maps


_NC_CACHE = {}


def get_nc():
    if "nc" not in _NC_CACHE:
        _NC_CACHE["nc"] = build_nc()
    return _NC_CACHE["nc"]


LAST_RESULTS = None  # BassKernelResults of the most recent kernel() call


def kernel(x, w_qkv, trace=False, **trace_kwargs):
    global LAST_RESULTS
    x = np.asarray(x)
    w_qkv = np.asarray(w_qkv)
    in_maps = make_in_maps(x, w_qkv)
    nc = get_nc()
    res = run_bass_kernel_spmd(nc, in_maps, list(range(N_CORES)),
                               trace=trace, **trace_kwargs)
    LAST_RESULTS = res
    out_hnd = np.empty((HEADS, N, HD), np.float32)
    for core in range(N_CORES):
        h, half = divmod(core, 2)
        out_hnd[h, half * NQ:(half + 1) * NQ, :] = res.results[core]["out"]
    return out_hnd.reshape(1, C, 16, 16, 16)
